# revision 53
# baseline (speedup 1.0000x reference)
import os
import sys

sys.path.insert(0, "/opt/trn_rl_repo")

import numpy as np
import ml_dtypes

import concourse.bacc as bacc
import concourse.bass as bass
import concourse.mybir as mybir
import concourse.tile as tile
from concourse.bass_utils import run_bass_kernel_spmd

F32 = mybir.dt.float32
BF16 = mybir.dt.bfloat16
BF = ml_dtypes.bfloat16

N, M, G, A, H = 20000, 48, 16, 64, 16
NCORES = 8
NL = N // NCORES      # 2500 atoms per core
NPAIR = NL // 2       # 1250 atom pairs
GJ = 256              # pairs per group (512 atoms)
KJ = 32               # pairs per input DMA chunk
TP = 8                # pairs per psum1 tile

_nc_cache = {}


def _build():
    """Per-core Bass program, bf16 PE pipeline, fp32 psum/output.

    Stage 1 (per atom-pair): one [128,128]x[128,64] matmul.  lhsT rows
    (x=parity 0:96, zero rows 96:128 for the K=128 FWL pad), cols
    (x', dgs, g) with parity-zeros baked in HBM.  psum1 rows
    (x, dgs, g): e: d0@0,d1@16,d2@32,S'@48; o: +64.  Copy -> vbig bf16.
    Stage 2 (q-tile = 4 channels x 256 pairs): 16 matmuls, K=128 full
    rows, lhsT [128,32] (cols (p,h), baked zeros select slot+parity),
    column tiles (0,32c).  psum2 rows (c,p,h), cols (slot, j).
    Finish: ACT square -> sq bf16; DVE adds (d-sum) -> ovin[s|v];
    PE transpose chunks -> psum_t[j, (c,p,h)]; copies assemble full
    output rows in out_asm; one fat 8KB-run DMA per half-group.
    """
    nc = bacc.Bacc("TRN2", target_bir_lowering=False)
    aw_d = nc.declare_dram_parameter("aw", [96, NPAIR, 128], BF16, isOutput=False)
    ar_d = nc.declare_dram_parameter("ar", [96, NPAIR, 64], BF16, isOutput=False)
    wg_d = nc.declare_dram_parameter("wg", [128, 8192], BF16, isOutput=False)
    id_d = nc.declare_dram_parameter("ident", [128, 128], BF16, isOutput=False)
    out_d = nc.declare_dram_parameter("out", [NL, A * G + A * H], F32, isOutput=True)

    Sq = mybir.ActivationFunctionType.Square
    # small leading groups shorten the DMA-bound pipeline fill
    sizes = [64, 192, 256, 256, 256, 226]
    assert sum(sizes) == NPAIR
    starts = [sum(sizes[:i]) for i in range(len(sizes))]
    ngroups = len(sizes)

    with tile.TileContext(nc) as tc:
        with (
            tc.tile_pool(name="singles", bufs=1) as singles,
            tc.tile_pool(name="lw", bufs=1) as lw_pool,
            tc.tile_pool(name="ar", bufs=1) as ar_pool,
            tc.tile_pool(name="vbig", bufs=1) as vbig_pool,
            tc.tile_pool(name="sq", bufs=2) as sq_pool,
            tc.tile_pool(name="ovin", bufs=16) as ovin_pool,
            tc.tile_pool(name="oasm", bufs=1) as oasm_pool,
            tc.tile_pool(name="psum1", bufs=2, space="PSUM") as p1_pool,
            tc.tile_pool(name="psum2", bufs=2, space="PSUM") as p2_pool,
            tc.tile_pool(name="psumt", bufs=2, space="PSUM") as pt_pool,
        ):
            wg = singles.tile([128, 8192], BF16)
            nc.scalar.dma_start(out=wg[:, :], in_=wg_d[:, :])
            ident = singles.tile([128, 128], BF16)
            nc.scalar.dma_start(out=ident[:, :], in_=id_d[:, :])

            # persistent rings: K-pad rows 96:128 must stay zero
            lws = [
                lw_pool.tile([128, KJ * 128], BF16, name=f"lw{i}") for i in range(3)
            ]
            ars = [
                ar_pool.tile([128, KJ * 64], BF16, name=f"arr{i}") for i in range(3)
            ]
            for lw in lws:
                nc.gpsimd.memset(lw[96:128, :], 0.0)
            for arr in ars:
                nc.gpsimd.memset(arr[96:128, :], 0.0)

            state = {"cglob": 0}

            def stage1(g2):
                j0 = starts[g2]
                jcnt = sizes[g2]
                vbig = vbig_pool.tile([128, A * GJ], BF16, name=f"vb{g2 % 2}")
                vb = vbig[:, :].rearrange("p (a j) -> p a j", j=GJ)

                nchunks = (jcnt + KJ - 1) // KJ
                for ck in range(nchunks):
                    cj0 = ck * KJ
                    cjc = min(KJ, jcnt - cj0)
                    Tg = j0 + cj0
                    lw = lws[state["cglob"] % 3]
                    arr = ars[state["cglob"] % 3]
                    state["cglob"] += 1
                    nc.sync.dma_start(
                        out=lw[0:96, 0 : 128 * cjc].rearrange(
                            "p (k z) -> p k z", z=128
                        ),
                        in_=aw_d[:, Tg : Tg + cjc, :],
                    )
                    nc.sync.dma_start(
                        out=arr[0:96, 0 : 64 * cjc].rearrange(
                            "p (k z) -> p k z", z=64
                        ),
                        in_=ar_d[:, Tg : Tg + cjc, :],
                    )
                    ntiles = (cjc + TP - 1) // TP
                    for t in range(ntiles):
                        tj0 = t * TP
                        tjc = min(TP, cjc - tj0)
                        psum1 = p1_pool.tile([128, 512], F32)
                        for k in range(tjc):
                            kk = tj0 + k
                            nc.tensor.matmul(
                                out=psum1[0:128, 64 * k : 64 * k + 64],
                                lhsT=lw[0:128, 128 * kk : 128 * kk + 128],
                                rhs=arr[0:128, 64 * kk : 64 * kk + 64],
                                start=True,
                                stop=True,
                            )
                        src = psum1[:, 0 : 64 * tjc].rearrange(
                            "p (k a) -> p a k", a=64
                        )
                        dst = vb[:, :, cj0 + tj0 : cj0 + tj0 + tjc]
                        if t % 3 == 0:
                            nc.vector.tensor_copy(out=dst, in_=src)
                        else:
                            nc.scalar.copy(out=dst, in_=src)
                return vbig

            def stage2_finish(g2, vbig):
                j0 = starts[g2]
                jcnt = sizes[g2]
                n0 = 2 * j0
                vb = vbig[:, :].rearrange("p (a j) -> p a j", j=GJ)
                # ---- stage 2 + finish part 1 ----
                ovins = []
                for q in range(16):
                    psum2 = p2_pool.tile([128, 1024], F32)
                    for s in range(4):
                        for c in range(4):
                            ch = 4 * q + c
                            c0 = 32 * (64 * s + ch)
                            nc.tensor.matmul(
                                out=psum2[32 * c : 32 * c + 32,
                                          256 * s : 256 * s + jcnt],
                                lhsT=wg[0:128, c0 : c0 + 32],
                                rhs=vb[0:128, ch, 0:jcnt],
                                start=True,
                                stop=True,
                                tile_position=(0, 32 * c),
                            )
                    sq = sq_pool.tile([128, 768], BF16)
                    nc.scalar.activation(
                        out=sq[:, :], in_=psum2[:, 0:768], func=Sq
                    )
                    ovin = ovin_pool.tile([128, 512], BF16)
                    ovins.append(ovin)
                    # ovin = [s-part 0:256 | v-part 256:512]
                    nc.gpsimd.tensor_add(
                        ovin[:, 256:512], sq[:, 0:256], sq[:, 256:512]
                    )
                    nc.gpsimd.tensor_add(
                        ovin[:, 256:512], ovin[:, 256:512], sq[:, 512:768]
                    )
                    nc.vector.tensor_copy(
                        out=ovin[:, 0:256], in_=psum2[:, 768:1024]
                    )

                # ---- finish part 2: transposes + assembly + out DMA ----
                # asm cols = (u=jh 2, p 2, w 2048); w = [s 1024 | v 1024]
                jh_sizes = [min(128, jcnt), max(0, jcnt - 128)]
                full = jh_sizes[0] == 128 and jh_sizes[1] == 128
                asm = oasm_pool.tile([128, 8192], F32, name="asm")
                for q in range(16):
                    ovin = ovins[q]
                    psum_t = pt_pool.tile([128, 512], BF16)
                    for vs in range(2):
                        for jh in range(2):
                            jhc = jh_sizes[jh]
                            if not jhc:
                                continue
                            nc.tensor.transpose(
                                out=psum_t[0:jhc,
                                           128 * (2 * vs + jh) :
                                           128 * (2 * vs + jh) + 128],
                                in_=ovin[:, 256 * vs + 128 * jh :
                                         256 * vs + 128 * jh + jhc],
                                identity=ident[:, :],
                            )
                    # ident is a permutation: transposed labels come out (p,c,h)
                    ptv = psum_t[:, :].rearrange(
                        "z (v u p w) -> z v u p w", v=2, u=2, p=2
                    )
                    asv = asm[:, :].rearrange(
                        "z (u p v q w) -> z u p v q w", u=2, p=2, v=2, q=16
                    )
                    if full:
                        # one 4-free-dim copy per vs, merging the jh halves
                        for vs in range(2):
                            src = ptv[:, vs]
                            dst = asv[:, :, :, vs, q]
                            if vs == 0:
                                nc.vector.tensor_copy(out=dst, in_=src)
                            else:
                                nc.scalar.copy(out=dst, in_=src)
                    else:
                        for vs in range(2):
                            for jh in range(2):
                                jhc = jh_sizes[jh]
                                if not jhc:
                                    continue
                                src = ptv[0:jhc, vs, jh]
                                dst = asv[0:jhc, jh, :, vs, q]
                                if vs == 0:
                                    nc.vector.tensor_copy(out=dst, in_=src)
                                else:
                                    nc.scalar.copy(out=dst, in_=src)
                for jh in range(2):
                    jhc = jh_sizes[jh]
                    if not jhc:
                        continue
                    r0 = n0 + 256 * jh
                    nc.sync.dma_start(
                        out=out_d[r0 : r0 + 2 * jhc, :].rearrange(
                            "(j p) w -> j p w", p=2
                        ),
                        in_=asm[0:jhc, :].rearrange(
                            "z (u p w) -> z u p w", u=2, p=2
                        )[:, jh],
                    )

            # software pipeline: stage1(g) overlaps stage2+finish(g-1)
            prev = None
            for g2 in range(ngroups):
                vb_t = stage1(g2)
                if prev is not None:
                    stage2_finish(g2 - 1, prev)
                prev = vb_t
            stage2_finish(ngroups - 1, prev)
    nc.compile()
    return nc


def _get_nc():
    if "nc" not in _nc_cache:
        _nc_cache["nc"] = _build()
    return _nc_cache["nc"]


def _prep(a, gs, gv, agh):
    """Host-side packing into the per-core HBM layouts (bf16)."""
    a = np.asarray(a, np.float32)
    gs = np.asarray(gs, np.float32)
    gv = np.asarray(gv, np.float32)
    agh = np.asarray(agh, np.float32)

    # weights per atom: [gv d0 | gv d1 | gv d2 | gs] (16 each) -> 64 cols
    wcat = np.empty((N, M, 64), dtype=BF)
    for d in range(3):
        wcat[:, :, 16 * d : 16 * d + 16] = gv[:, :, :, d].astype(BF)
    wcat[:, :, 48:64] = gs.astype(BF)
    a16 = a.astype(BF)

    # aw[core][r=(x,m), j, 64x:64x+64] = wcat[n0+2j+x, m]; zeros elsewhere
    aw = np.zeros((NCORES, 96, NPAIR, 128), dtype=BF)
    ar = np.empty((NCORES, 96, NPAIR, 64), dtype=BF)
    wc = wcat.reshape(NCORES, NPAIR, 2, M, 64)
    ac = a16.reshape(NCORES, NPAIR, 2, M, 64)
    for x in range(2):
        aw[:, 48 * x : 48 * x + 48, :, 64 * x : 64 * x + 64] = wc[
            :, :, x
        ].transpose(0, 2, 1, 3)
        ar[:, 48 * x : 48 * x + 48] = ac[:, :, x].transpose(0, 2, 1, 3)

    # stage-2 weights: block (s, ch) = [128, 32], cols (p 2, h 16), value
    # agh[ch][g, h] (ident for s=3) at K-rows 64p+16s+g, zeros elsewhere
    wgm = np.zeros((128, 8192), dtype=BF)
    aghT = agh.transpose(1, 0, 2).astype(BF)  # [g, a, h]
    eye = np.eye(16, dtype=BF)
    for s in range(4):
        for ch in range(A):
            c0 = 32 * (64 * s + ch)
            blk = eye if s == 3 else aghT[:, ch, :]
            for p in range(2):
                r0 = 64 * p + 16 * s
                wgm[r0 : r0 + 16, c0 + 16 * p : c0 + 16 * p + 16] = blk

    # permutation for the PE transpose: label (c,p,h)=32c+16p+h goes to
    # output column (p,c,h)=64p+16c+h
    ident = np.zeros((128, 128), dtype=BF)
    for c in range(4):
        for p in range(2):
            for h in range(16):
                ident[32 * c + 16 * p + h, 64 * p + 16 * c + h] = 1
    return aw, ar, wgm, ident


def _in_maps(inputs):
    aw, ar, wgm, ident = _prep(
        inputs["a"], inputs["gs"], inputs["gv"], inputs["agh"]
    )
    return [
        {"aw": aw[c], "ar": ar[c], "wg": wgm, "ident": ident}
        for c in range(NCORES)
    ]


def kernel(a, gs, gv, agh):
    nc = _get_nc()
    in_maps = _in_maps({"a": a, "gs": gs, "gv": gv, "agh": agh})
    res = run_bass_kernel_spmd(nc, in_maps, list(range(NCORES))).results
    return np.concatenate([res[c]["out"] for c in range(NCORES)], axis=0)


# revision 54
# speedup vs baseline: 1.0169x; 1.0169x over previous
import os
import sys

sys.path.insert(0, "/opt/trn_rl_repo")

import numpy as np
import ml_dtypes

import concourse.bacc as bacc
import concourse.bass as bass
import concourse.mybir as mybir
import concourse.tile as tile
from concourse.bass_utils import run_bass_kernel_spmd

F32 = mybir.dt.float32
BF16 = mybir.dt.bfloat16
BF = ml_dtypes.bfloat16

N, M, G, A, H = 20000, 48, 16, 64, 16
NCORES = 8
NL = N // NCORES      # 2500 atoms per core
NPAIR = NL // 2       # 1250 atom pairs
GJ = 256              # pairs per group (512 atoms)
KJ = 32               # pairs per input DMA chunk
TP = 8                # pairs per psum1 tile

_nc_cache = {}


def _build():
    """Per-core Bass program, bf16 PE pipeline, fp32 psum/output.

    Stage 1 (per atom-pair): one [128,128]x[128,64] matmul.  lhsT rows
    (x=parity 0:96, zero rows 96:128 for the K=128 FWL pad), cols
    (x', dgs, g) with parity-zeros baked in HBM.  psum1 rows
    (x, dgs, g): e: d0@0,d1@16,d2@32,S'@48; o: +64.  Copy -> vbig bf16.
    Stage 2 (q-tile = 4 channels x 256 pairs): 16 matmuls, K=128 full
    rows, lhsT [128,32] (cols (p,h), baked zeros select slot+parity),
    column tiles (0,32c).  psum2 rows (c,p,h), cols (slot, j).
    Finish: ACT square -> sq bf16; DVE adds (d-sum) -> ovin[s|v];
    PE transpose chunks -> psum_t[j, (c,p,h)]; copies assemble full
    output rows in out_asm; one fat 8KB-run DMA per half-group.
    """
    nc = bacc.Bacc("TRN2", target_bir_lowering=False)
    aw_d = nc.declare_dram_parameter("aw", [96, NPAIR, 128], BF16, isOutput=False)
    ar_d = nc.declare_dram_parameter("ar", [96, NPAIR, 64], BF16, isOutput=False)
    wg_d = nc.declare_dram_parameter("wg", [128, 8192], BF16, isOutput=False)
    id_d = nc.declare_dram_parameter("ident", [128, 128], BF16, isOutput=False)
    out_d = nc.declare_dram_parameter("out", [NL, A * G + A * H], F32, isOutput=True)

    Sq = mybir.ActivationFunctionType.Square
    # small leading groups shorten the DMA-bound pipeline fill
    sizes = [64, 192, 256, 256, 256, 226]
    assert sum(sizes) == NPAIR
    starts = [sum(sizes[:i]) for i in range(len(sizes))]
    ngroups = len(sizes)

    with tile.TileContext(nc) as tc:
        with (
            tc.tile_pool(name="singles", bufs=1) as singles,
            tc.tile_pool(name="lw", bufs=1) as lw_pool,
            tc.tile_pool(name="ar", bufs=1) as ar_pool,
            tc.tile_pool(name="vbig", bufs=1) as vbig_pool,
            tc.tile_pool(name="sq", bufs=2) as sq_pool,
            tc.tile_pool(name="ovin", bufs=16) as ovin_pool,
            tc.tile_pool(name="oasm", bufs=1) as oasm_pool,
            tc.tile_pool(name="psum1", bufs=2, space="PSUM") as p1_pool,
            tc.tile_pool(name="psum2", bufs=2, space="PSUM") as p2_pool,
            tc.tile_pool(name="psumt", bufs=2, space="PSUM") as pt_pool,
        ):
            wg = singles.tile([128, 8192], BF16)
            nc.scalar.dma_start(out=wg[:, :], in_=wg_d[:, :])
            ident = singles.tile([128, 128], BF16)
            nc.scalar.dma_start(out=ident[:, :], in_=id_d[:, :])

            # persistent rings: K-pad rows 96:128 must stay zero
            lws = [
                lw_pool.tile([128, KJ * 128], BF16, name=f"lw{i}") for i in range(3)
            ]
            ars = [
                ar_pool.tile([128, KJ * 64], BF16, name=f"arr{i}") for i in range(3)
            ]
            for lw in lws:
                nc.gpsimd.memset(lw[96:128, :], 0.0)
            for arr in ars:
                nc.gpsimd.memset(arr[96:128, :], 0.0)

            state = {"cglob": 0}

            def stage1(g2):
                j0 = starts[g2]
                jcnt = sizes[g2]
                vbig = vbig_pool.tile([128, A * GJ], BF16, name=f"vb{g2 % 2}")
                vb = vbig[:, :].rearrange("p (a j) -> p a j", j=GJ)

                nchunks = (jcnt + KJ - 1) // KJ
                for ck in range(nchunks):
                    cj0 = ck * KJ
                    cjc = min(KJ, jcnt - cj0)
                    Tg = j0 + cj0
                    lw = lws[state["cglob"] % 3]
                    arr = ars[state["cglob"] % 3]
                    state["cglob"] += 1
                    nc.sync.dma_start(
                        out=lw[0:96, 0 : 128 * cjc].rearrange(
                            "p (k z) -> p k z", z=128
                        ),
                        in_=aw_d[:, Tg : Tg + cjc, :],
                    )
                    nc.sync.dma_start(
                        out=arr[0:96, 0 : 64 * cjc].rearrange(
                            "p (k z) -> p k z", z=64
                        ),
                        in_=ar_d[:, Tg : Tg + cjc, :],
                    )
                    ntiles = (cjc + TP - 1) // TP
                    for t in range(ntiles):
                        tj0 = t * TP
                        tjc = min(TP, cjc - tj0)
                        psum1 = p1_pool.tile([128, 512], F32)
                        for k in range(tjc):
                            kk = tj0 + k
                            nc.tensor.matmul(
                                out=psum1[0:128, 64 * k : 64 * k + 64],
                                lhsT=lw[0:128, 128 * kk : 128 * kk + 128],
                                rhs=arr[0:128, 64 * kk : 64 * kk + 64],
                                start=True,
                                stop=True,
                            )
                        src = psum1[:, 0 : 64 * tjc].rearrange(
                            "p (k a) -> p a k", a=64
                        )
                        dst = vb[:, :, cj0 + tj0 : cj0 + tj0 + tjc]
                        if t % 2 == 0:
                            nc.vector.tensor_copy(out=dst, in_=src)
                        else:
                            nc.scalar.copy(out=dst, in_=src)
                return vbig

            def stage2_finish(g2, vbig):
                j0 = starts[g2]
                jcnt = sizes[g2]
                n0 = 2 * j0
                vb = vbig[:, :].rearrange("p (a j) -> p a j", j=GJ)
                # ---- stage 2 + finish part 1 ----
                ovins = []
                for q in range(16):
                    psum2 = p2_pool.tile([128, 1024], F32)
                    for s in range(4):
                        for c in range(4):
                            ch = 4 * q + c
                            c0 = 32 * (64 * s + ch)
                            nc.tensor.matmul(
                                out=psum2[32 * c : 32 * c + 32,
                                          256 * s : 256 * s + jcnt],
                                lhsT=wg[0:128, c0 : c0 + 32],
                                rhs=vb[0:128, ch, 0:jcnt],
                                start=True,
                                stop=True,
                                tile_position=(0, 32 * c),
                            )
                    sq = sq_pool.tile([128, 768], BF16)
                    nc.scalar.activation(
                        out=sq[:, :], in_=psum2[:, 0:768], func=Sq
                    )
                    ovin = ovin_pool.tile([128, 512], BF16)
                    ovins.append(ovin)
                    # ovin = [s-part 0:256 | v-part 256:512]
                    nc.vector.tensor_add(
                        ovin[:, 256:512], sq[:, 0:256], sq[:, 256:512]
                    )
                    nc.vector.tensor_add(
                        ovin[:, 256:512], ovin[:, 256:512], sq[:, 512:768]
                    )
                    nc.scalar.copy(out=ovin[:, 0:256], in_=psum2[:, 768:1024])

                # ---- finish part 2: transposes + assembly + out DMA ----
                # asm cols = (u=jh 2, p 2, w 2048); w = [s 1024 | v 1024]
                jh_sizes = [min(128, jcnt), max(0, jcnt - 128)]
                full = jh_sizes[0] == 128 and jh_sizes[1] == 128
                asm = oasm_pool.tile([128, 8192], F32, name="asm")
                for q in range(16):
                    ovin = ovins[q]
                    psum_t = pt_pool.tile([128, 512], BF16)
                    for vs in range(2):
                        for jh in range(2):
                            jhc = jh_sizes[jh]
                            if not jhc:
                                continue
                            nc.tensor.transpose(
                                out=psum_t[0:jhc,
                                           128 * (2 * vs + jh) :
                                           128 * (2 * vs + jh) + 128],
                                in_=ovin[:, 256 * vs + 128 * jh :
                                         256 * vs + 128 * jh + jhc],
                                identity=ident[:, :],
                            )
                    # ident is a permutation: transposed labels come out (p,c,h)
                    ptv = psum_t[:, :].rearrange(
                        "z (v u p w) -> z v u p w", v=2, u=2, p=2
                    )
                    asv = asm[:, :].rearrange(
                        "z (u p v q w) -> z u p v q w", u=2, p=2, v=2, q=16
                    )
                    if full:
                        # one 4-free-dim copy per vs, merging the jh halves
                        for vs in range(2):
                            src = ptv[:, vs]
                            dst = asv[:, :, :, vs, q]
                            if q % 2 == 0:
                                nc.vector.tensor_copy(out=dst, in_=src)
                            else:
                                nc.scalar.copy(out=dst, in_=src)
                    else:
                        for vs in range(2):
                            for jh in range(2):
                                jhc = jh_sizes[jh]
                                if not jhc:
                                    continue
                                src = ptv[0:jhc, vs, jh]
                                dst = asv[0:jhc, jh, :, vs, q]
                                if q % 2 == 0:
                                    nc.vector.tensor_copy(out=dst, in_=src)
                                else:
                                    nc.scalar.copy(out=dst, in_=src)
                for jh in range(2):
                    jhc = jh_sizes[jh]
                    if not jhc:
                        continue
                    r0 = n0 + 256 * jh
                    nc.sync.dma_start(
                        out=out_d[r0 : r0 + 2 * jhc, :].rearrange(
                            "(j p) w -> j p w", p=2
                        ),
                        in_=asm[0:jhc, :].rearrange(
                            "z (u p w) -> z u p w", u=2, p=2
                        )[:, jh],
                    )

            # software pipeline: stage1(g) overlaps stage2+finish(g-1)
            prev = None
            for g2 in range(ngroups):
                vb_t = stage1(g2)
                if prev is not None:
                    stage2_finish(g2 - 1, prev)
                prev = vb_t
            stage2_finish(ngroups - 1, prev)
    nc.compile()
    return nc


def _get_nc():
    if "nc" not in _nc_cache:
        _nc_cache["nc"] = _build()
    return _nc_cache["nc"]


def _prep(a, gs, gv, agh):
    """Host-side packing into the per-core HBM layouts (bf16)."""
    a = np.asarray(a, np.float32)
    gs = np.asarray(gs, np.float32)
    gv = np.asarray(gv, np.float32)
    agh = np.asarray(agh, np.float32)

    # weights per atom: [gv d0 | gv d1 | gv d2 | gs] (16 each) -> 64 cols
    wcat = np.empty((N, M, 64), dtype=BF)
    for d in range(3):
        wcat[:, :, 16 * d : 16 * d + 16] = gv[:, :, :, d].astype(BF)
    wcat[:, :, 48:64] = gs.astype(BF)
    a16 = a.astype(BF)

    # aw[core][r=(x,m), j, 64x:64x+64] = wcat[n0+2j+x, m]; zeros elsewhere
    aw = np.zeros((NCORES, 96, NPAIR, 128), dtype=BF)
    ar = np.empty((NCORES, 96, NPAIR, 64), dtype=BF)
    wc = wcat.reshape(NCORES, NPAIR, 2, M, 64)
    ac = a16.reshape(NCORES, NPAIR, 2, M, 64)
    for x in range(2):
        aw[:, 48 * x : 48 * x + 48, :, 64 * x : 64 * x + 64] = wc[
            :, :, x
        ].transpose(0, 2, 1, 3)
        ar[:, 48 * x : 48 * x + 48] = ac[:, :, x].transpose(0, 2, 1, 3)

    # stage-2 weights: block (s, ch) = [128, 32], cols (p 2, h 16), value
    # agh[ch][g, h] (ident for s=3) at K-rows 64p+16s+g, zeros elsewhere
    wgm = np.zeros((128, 8192), dtype=BF)
    aghT = agh.transpose(1, 0, 2).astype(BF)  # [g, a, h]
    eye = np.eye(16, dtype=BF)
    for s in range(4):
        for ch in range(A):
            c0 = 32 * (64 * s + ch)
            blk = eye if s == 3 else aghT[:, ch, :]
            for p in range(2):
                r0 = 64 * p + 16 * s
                wgm[r0 : r0 + 16, c0 + 16 * p : c0 + 16 * p + 16] = blk

    # permutation for the PE transpose: label (c,p,h)=32c+16p+h goes to
    # output column (p,c,h)=64p+16c+h
    ident = np.zeros((128, 128), dtype=BF)
    for c in range(4):
        for p in range(2):
            for h in range(16):
                ident[32 * c + 16 * p + h, 64 * p + 16 * c + h] = 1
    return aw, ar, wgm, ident


def _in_maps(inputs):
    aw, ar, wgm, ident = _prep(
        inputs["a"], inputs["gs"], inputs["gv"], inputs["agh"]
    )
    return [
        {"aw": aw[c], "ar": ar[c], "wg": wgm, "ident": ident}
        for c in range(NCORES)
    ]


def kernel(a, gs, gv, agh):
    nc = _get_nc()
    in_maps = _in_maps({"a": a, "gs": gs, "gv": gv, "agh": agh})
    res = run_bass_kernel_spmd(nc, in_maps, list(range(NCORES))).results
    return np.concatenate([res[c]["out"] for c in range(NCORES)], axis=0)


# revision 55
# speedup vs baseline: 1.0605x; 1.0429x over previous
import os
import sys

sys.path.insert(0, "/opt/trn_rl_repo")

import numpy as np
import ml_dtypes

import concourse.bacc as bacc
import concourse.bass as bass
import concourse.mybir as mybir
import concourse.tile as tile
from concourse.bass_utils import run_bass_kernel_spmd

F32 = mybir.dt.float32
BF16 = mybir.dt.bfloat16
BF = ml_dtypes.bfloat16

N, M, G, A, H = 20000, 48, 16, 64, 16
NCORES = 8
NL = N // NCORES      # 2500 atoms per core
NPAIR = NL // 2       # 1250 atom pairs
GJ = 256              # pairs per group (512 atoms)
KJ = 32               # pairs per input DMA chunk
TP = 8                # pairs per psum1 tile

_nc_cache = {}


def _build():
    """Per-core Bass program, bf16 PE pipeline, fp32 psum/output.

    Stage 1 (per atom-pair): one [128,128]x[128,64] matmul.  lhsT rows
    (x=parity 0:96, zero rows 96:128 for the K=128 FWL pad), cols
    (x', dgs, g) with parity-zeros baked in HBM.  psum1 rows
    (x, dgs, g): e: d0@0,d1@16,d2@32,S'@48; o: +64.  Copy -> vbig bf16.
    Stage 2 (q-tile = 4 channels x 256 pairs): 16 matmuls, K=128 full
    rows, lhsT [128,32] (cols (p,h), baked zeros select slot+parity),
    column tiles (0,32c).  psum2 rows (c,p,h), cols (slot, j).
    Finish: ACT square -> sq bf16; DVE adds (d-sum) -> ovin[s|v];
    PE transpose chunks -> psum_t[j, (c,p,h)]; copies assemble full
    output rows in out_asm; one fat 8KB-run DMA per half-group.
    """
    nc = bacc.Bacc("TRN2", target_bir_lowering=False)
    aw_d = nc.declare_dram_parameter("aw", [96, NPAIR, 128], BF16, isOutput=False)
    ar_d = nc.declare_dram_parameter("ar", [96, NPAIR, 64], BF16, isOutput=False)
    wg_d = nc.declare_dram_parameter("wg", [128, 8192], BF16, isOutput=False)
    id_d = nc.declare_dram_parameter("ident", [128, 128], BF16, isOutput=False)
    out_d = nc.declare_dram_parameter("out", [NL, A * G + A * H], F32, isOutput=True)

    Sq = mybir.ActivationFunctionType.Square
    # small leading groups shorten the DMA-bound pipeline fill
    sizes = [256, 256, 256, 256, 226]
    assert sum(sizes) == NPAIR
    starts = [sum(sizes[:i]) for i in range(len(sizes))]
    ngroups = len(sizes)

    with tile.TileContext(nc) as tc:
        with (
            tc.tile_pool(name="singles", bufs=1) as singles,
            tc.tile_pool(name="lw", bufs=1) as lw_pool,
            tc.tile_pool(name="ar", bufs=1) as ar_pool,
            tc.tile_pool(name="vbig", bufs=1) as vbig_pool,
            tc.tile_pool(name="sq", bufs=2) as sq_pool,
            tc.tile_pool(name="ovin", bufs=16) as ovin_pool,
            tc.tile_pool(name="oasm", bufs=1) as oasm_pool,
            tc.tile_pool(name="psum1", bufs=2, space="PSUM") as p1_pool,
            tc.tile_pool(name="psum2", bufs=2, space="PSUM") as p2_pool,
            tc.tile_pool(name="psumt", bufs=2, space="PSUM") as pt_pool,
        ):
            wg = singles.tile([128, 8192], BF16)
            nc.sync.dma_start(out=wg[:, :], in_=wg_d[:, :])
            ident = singles.tile([128, 128], BF16)
            nc.sync.dma_start(out=ident[:, :], in_=id_d[:, :])

            # persistent rings: K-pad rows 96:128 must stay zero
            lws = [
                lw_pool.tile([128, KJ * 128], BF16, name=f"lw{i}") for i in range(3)
            ]
            ars = [
                ar_pool.tile([128, KJ * 64], BF16, name=f"arr{i}") for i in range(3)
            ]
            for lw in lws:
                nc.gpsimd.memset(lw[96:128, :], 0.0)
            for arr in ars:
                nc.gpsimd.memset(arr[96:128, :], 0.0)

            state = {"cglob": 0}

            def stage1(g2):
                j0 = starts[g2]
                jcnt = sizes[g2]
                vbig = vbig_pool.tile([128, A * GJ], BF16, name=f"vb{g2 % 2}")
                vb = vbig[:, :].rearrange("p (a j) -> p a j", j=GJ)

                nchunks = (jcnt + KJ - 1) // KJ
                for ck in range(nchunks):
                    cj0 = ck * KJ
                    cjc = min(KJ, jcnt - cj0)
                    Tg = j0 + cj0
                    lw = lws[state["cglob"] % 3]
                    arr = ars[state["cglob"] % 3]
                    state["cglob"] += 1
                    nc.sync.dma_start(
                        out=lw[0:96, 0 : 128 * cjc].rearrange(
                            "p (k z) -> p k z", z=128
                        ),
                        in_=aw_d[:, Tg : Tg + cjc, :],
                    )
                    nc.sync.dma_start(
                        out=arr[0:96, 0 : 64 * cjc].rearrange(
                            "p (k z) -> p k z", z=64
                        ),
                        in_=ar_d[:, Tg : Tg + cjc, :],
                    )
                    ntiles = (cjc + TP - 1) // TP
                    for t in range(ntiles):
                        tj0 = t * TP
                        tjc = min(TP, cjc - tj0)
                        psum1 = p1_pool.tile([128, 512], F32)
                        for k in range(tjc):
                            kk = tj0 + k
                            nc.tensor.matmul(
                                out=psum1[0:128, 64 * k : 64 * k + 64],
                                lhsT=lw[0:128, 128 * kk : 128 * kk + 128],
                                rhs=arr[0:128, 64 * kk : 64 * kk + 64],
                                start=True,
                                stop=True,
                            )
                        src = psum1[:, 0 : 64 * tjc].rearrange(
                            "p (k a) -> p a k", a=64
                        )
                        dst = vb[:, :, cj0 + tj0 : cj0 + tj0 + tjc]
                        if t % 2 == 0:
                            nc.vector.tensor_copy(out=dst, in_=src)
                        else:
                            nc.scalar.copy(out=dst, in_=src)
                return vbig

            def stage2_finish(g2, vbig):
                j0 = starts[g2]
                jcnt = sizes[g2]
                n0 = 2 * j0
                vb = vbig[:, :].rearrange("p (a j) -> p a j", j=GJ)
                # ---- stage 2 + finish part 1 ----
                ovins = []
                for q in range(16):
                    psum2 = p2_pool.tile([128, 1024], F32)
                    for s in range(4):
                        for c in range(4):
                            ch = 4 * q + c
                            c0 = 32 * (64 * s + ch)
                            nc.tensor.matmul(
                                out=psum2[32 * c : 32 * c + 32,
                                          256 * s : 256 * s + jcnt],
                                lhsT=wg[0:128, c0 : c0 + 32],
                                rhs=vb[0:128, ch, 0:jcnt],
                                start=True,
                                stop=True,
                                tile_position=(0, 32 * c),
                            )
                    sq = sq_pool.tile([128, 768], BF16)
                    nc.scalar.activation(
                        out=sq[:, :], in_=psum2[:, 0:768], func=Sq
                    )
                    ovin = ovin_pool.tile([128, 512], BF16)
                    ovins.append(ovin)
                    # ovin = [s-part 0:256 | v-part 256:512]
                    nc.vector.tensor_add(
                        ovin[:, 256:512], sq[:, 0:256], sq[:, 256:512]
                    )
                    nc.vector.tensor_add(
                        ovin[:, 256:512], ovin[:, 256:512], sq[:, 512:768]
                    )
                    nc.scalar.copy(out=ovin[:, 0:256], in_=psum2[:, 768:1024])

                # ---- finish part 2: transposes + assembly + out DMA ----
                # asm cols = (u=jh 2, p 2, w 2048); w = [s 1024 | v 1024]
                jh_sizes = [min(128, jcnt), max(0, jcnt - 128)]
                full = jh_sizes[0] == 128 and jh_sizes[1] == 128
                asm = oasm_pool.tile([128, 8192], F32, name="asm")
                for q in range(16):
                    ovin = ovins[q]
                    psum_t = pt_pool.tile([128, 512], BF16)
                    for vs in range(2):
                        for jh in range(2):
                            jhc = jh_sizes[jh]
                            if not jhc:
                                continue
                            nc.tensor.transpose(
                                out=psum_t[0:jhc,
                                           128 * (2 * vs + jh) :
                                           128 * (2 * vs + jh) + 128],
                                in_=ovin[:, 256 * vs + 128 * jh :
                                         256 * vs + 128 * jh + jhc],
                                identity=ident[:, :],
                            )
                    # ident is a permutation: transposed labels come out (p,c,h)
                    ptv = psum_t[:, :].rearrange(
                        "z (v u p w) -> z v u p w", v=2, u=2, p=2
                    )
                    asv = asm[:, :].rearrange(
                        "z (u p v q w) -> z u p v q w", u=2, p=2, v=2, q=16
                    )
                    if full:
                        # one 4-free-dim copy per vs, merging the jh halves
                        for vs in range(2):
                            src = ptv[:, vs]
                            dst = asv[:, :, :, vs, q]
                            if q % 2 == 0:
                                nc.vector.tensor_copy(out=dst, in_=src)
                            else:
                                nc.scalar.copy(out=dst, in_=src)
                    else:
                        for vs in range(2):
                            for jh in range(2):
                                jhc = jh_sizes[jh]
                                if not jhc:
                                    continue
                                src = ptv[0:jhc, vs, jh]
                                dst = asv[0:jhc, jh, :, vs, q]
                                if q % 2 == 0:
                                    nc.vector.tensor_copy(out=dst, in_=src)
                                else:
                                    nc.scalar.copy(out=dst, in_=src)
                for jh in range(2):
                    jhc = jh_sizes[jh]
                    if not jhc:
                        continue
                    r0 = n0 + 256 * jh
                    nc.sync.dma_start(
                        out=out_d[r0 : r0 + 2 * jhc, :].rearrange(
                            "(j p) w -> j p w", p=2
                        ),
                        in_=asm[0:jhc, :].rearrange(
                            "z (u p w) -> z u p w", u=2, p=2
                        )[:, jh],
                    )

            # software pipeline: stage1(g) overlaps stage2+finish(g-1)
            prev = None
            for g2 in range(ngroups):
                vb_t = stage1(g2)
                if prev is not None:
                    stage2_finish(g2 - 1, prev)
                prev = vb_t
            stage2_finish(ngroups - 1, prev)
    nc.compile()
    return nc


def _get_nc():
    if "nc" not in _nc_cache:
        _nc_cache["nc"] = _build()
    return _nc_cache["nc"]


def _prep(a, gs, gv, agh):
    """Host-side packing into the per-core HBM layouts (bf16)."""
    a = np.asarray(a, np.float32)
    gs = np.asarray(gs, np.float32)
    gv = np.asarray(gv, np.float32)
    agh = np.asarray(agh, np.float32)

    # weights per atom: [gv d0 | gv d1 | gv d2 | gs] (16 each) -> 64 cols
    wcat = np.empty((N, M, 64), dtype=BF)
    for d in range(3):
        wcat[:, :, 16 * d : 16 * d + 16] = gv[:, :, :, d].astype(BF)
    wcat[:, :, 48:64] = gs.astype(BF)
    a16 = a.astype(BF)

    # aw[core][r=(x,m), j, 64x:64x+64] = wcat[n0+2j+x, m]; zeros elsewhere
    aw = np.zeros((NCORES, 96, NPAIR, 128), dtype=BF)
    ar = np.empty((NCORES, 96, NPAIR, 64), dtype=BF)
    wc = wcat.reshape(NCORES, NPAIR, 2, M, 64)
    ac = a16.reshape(NCORES, NPAIR, 2, M, 64)
    for x in range(2):
        aw[:, 48 * x : 48 * x + 48, :, 64 * x : 64 * x + 64] = wc[
            :, :, x
        ].transpose(0, 2, 1, 3)
        ar[:, 48 * x : 48 * x + 48] = ac[:, :, x].transpose(0, 2, 1, 3)

    # stage-2 weights: block (s, ch) = [128, 32], cols (p 2, h 16), value
    # agh[ch][g, h] (ident for s=3) at K-rows 64p+16s+g, zeros elsewhere
    wgm = np.zeros((128, 8192), dtype=BF)
    aghT = agh.transpose(1, 0, 2).astype(BF)  # [g, a, h]
    eye = np.eye(16, dtype=BF)
    for s in range(4):
        for ch in range(A):
            c0 = 32 * (64 * s + ch)
            blk = eye if s == 3 else aghT[:, ch, :]
            for p in range(2):
                r0 = 64 * p + 16 * s
                wgm[r0 : r0 + 16, c0 + 16 * p : c0 + 16 * p + 16] = blk

    # permutation for the PE transpose: label (c,p,h)=32c+16p+h goes to
    # output column (p,c,h)=64p+16c+h
    ident = np.zeros((128, 128), dtype=BF)
    for c in range(4):
        for p in range(2):
            for h in range(16):
                ident[32 * c + 16 * p + h, 64 * p + 16 * c + h] = 1
    return aw, ar, wgm, ident


def _in_maps(inputs):
    aw, ar, wgm, ident = _prep(
        inputs["a"], inputs["gs"], inputs["gv"], inputs["agh"]
    )
    return [
        {"aw": aw[c], "ar": ar[c], "wg": wgm, "ident": ident}
        for c in range(NCORES)
    ]


def kernel(a, gs, gv, agh):
    nc = _get_nc()
    in_maps = _in_maps({"a": a, "gs": gs, "gv": gv, "agh": agh})
    res = run_bass_kernel_spmd(nc, in_maps, list(range(NCORES))).results
    return np.concatenate([res[c]["out"] for c in range(NCORES)], axis=0)
